# revision 16
# baseline (speedup 1.0000x reference)
"""Trainium2 Bass kernel for the MANTIS quantum-circuit-loss nn.Module.

Shapes (hardcoded): B=128, L=16, M=32, P=4.  8 NeuronCores, batch-sharded
(16 batch elements per core).

Math
----
Let j = (m, p) flattened (M*P = 128 == partition count) and
    A[b, l, j] = theta[l, j] + scal[p(j)] * input_ds[b, l]
    CA = cos(A), SA = sin(A)                       (ACT Sin + pi/2 bias)

prob term:      amp[b]  = sum_j coef_j prod_l CA[b,l,j]
normalization:  norm[b] = sum_{j,k} coef_j coef_k prod_l cos(A[b,l,j]-A[b,l,k])

Using cos(a-b) = cos a cos b + sin a sin b, norm[b] is the squared norm of a
sum of 128 product states in the 2^16-dim site space.  Split the 16 sites
into two groups of 8; for each group build the 256 branch-product vectors
    U_g[j, T] = prod_{l in g} X_{T_l}[b, l, j],  X_0 = CA, X_1 = SA
by log-doubling (elementwise multiplies, bf16).  With coef folded into U1:
    D_b[T1, T2] = sum_j (c U1)[j, T1] U2[j, T2]    (PE matmul, K = 128)
    norm[b] = sum_{T1,T2} D_b^2
    amp[b]  = D_b[0, 0]
The device emits per-(partition, b) partials fin[128, 32]
(cols 0:16 = norm partials, 16:32 = amp partials); the host finishes:
partition sums, ln, mean over b, and the tiny regularization variances.

input_ds rides in as a single [1, 256] row and is broadcast to all 128
partitions with a K=1 f32r ones-matmul (cheaper than DMAing the
broadcast).  Engine split: ACT does cos/sin + PSUM square passes; DVE
does stage A + group-0 doubling + its share of square passes; Pool
(nc.gpsimd) does group-1 doubling and reduce second-passes.
"""

import math
import os

import numpy as np

import concourse.bacc as bacc
import concourse.bass as bass
import concourse.mybir as mybir
import concourse.tile as tile

B, L, M, P = 128, 16, 32, 4
NCORES = 8
BLOC = B // NCORES  # 16 batch elements per core
J = M * P  # 128
EPS = 1e-20
REG_C = 0.01
REG_THETA_M = 0.01
REG_THETA_P = 0.01

F32 = mybir.dt.float32
F32R = mybir.dt.float32r
BF16 = mybir.dt.bfloat16
AF = mybir.ActivationFunctionType
ALU = mybir.AluOpType

# batch chunk sizes for the U-build / matmul / square pipeline
CHUNKS = [int(x) for x in os.environ.get("MANTIS_CHUNKS", "1,1,2,4,4,4").split(",")]
# per-b square-reduce mode (16 chars):
#   A = ACT Square+accum direct from PSUM (1 pass)
#   S = ACT Square -> SBUF bf16, Pool tensor_reduce XYZWC -> fin[0, i]
#   Q = ACT Square -> SBUF bf16, DVE sums (tensor_scalar accum)
#   W = DVE copy -> SBUF bf16, DVE squares+sums (STT accum)
#   H = column-split: ACT squares cols 0:256, DVE copy+squares cols 256:512
#       (second half accumulates into fin[:, 32 + (i-14)]; only i=14,15)
SQ_MODES = os.environ.get("MANTIS_SQ", "AAAQAAAQAAAQAAAA")
# how many U chunks of group 1 go to Pool (from chunk 0 upward)
N_UPOOL = int(os.environ.get("MANTIS_UPOOL", "0"))
# doubling group-1 engine: pool | dve
G1_ENG = os.environ.get("MANTIS_G1ENG", "pool")

# params column layout
PC_THETA = 0  # 16 cols: theta_t[j, l]
PC_COEF = 16  # 1 col
PC_SCAL = 17  # 1 col: pi / 2^(p(j)+1)
PC_HALFPI = 18  # 1 col: pi/2 (ACT bias for cos-via-sin)
P_COLS = 19

# fin layout: [J, 34]: 0:16 norm partials, 16:32 amp partials, 32:34 H halves
FIN_COLS = 34


def build_params() -> np.ndarray:
    pr = np.zeros((J, P_COLS), dtype=np.float32)
    sf = (np.pi / 2.0 ** (np.arange(P) + 1.0)).astype(np.float32)
    pr[:, PC_SCAL] = np.tile(sf, M)
    pr[:, PC_HALFPI] = np.pi / 2.0
    return pr


def build_program():
    """Build the SPMD Bass/Tile program (identical on all 8 cores)."""
    nc = bacc.Bacc(
        "TRN2",
        target_bir_lowering=False,
        debug=False,
        num_devices=NCORES,
    )
    params_d = nc.dram_tensor("params", [J, P_COLS], F32, kind="ExternalInput")
    inds_d = nc.dram_tensor("inds", [1, BLOC * L], F32R, kind="ExternalInput")
    out_d = nc.dram_tensor("out", [J, FIN_COLS], F32, kind="ExternalOutput")

    with tile.TileContext(nc) as tc:
        with (
            tc.tile_pool(name="const", bufs=1) as cpool,
            tc.tile_pool(name="work", bufs=1) as wpool,
            tc.tile_pool(name="dps", bufs=6, space=bass.MemorySpace.PSUM) as dpool,
            tc.tile_pool(name="bps", bufs=1, space=bass.MemorySpace.PSUM) as bpool,
            tc.tile_pool(name="dsqp", bufs=3) as spool,
        ):
            _emit(nc, tc, cpool, wpool, dpool, bpool, spool, params_d, inds_d, out_d)
    nc.compile()
    return nc


def _emit(nc, tc, cpool, wpool, dpool, bpool, spool, params_d, inds_d, out_d):
    params = cpool.tile([J, P_COLS], F32, tag="params")
    inds_row = cpool.tile([1, BLOC * L], F32R, tag="inds_row")
    nc.sync.dma_start(inds_row[:], inds_d[:, :])
    nc.sync.dma_start(params[:], params_d[:, :])

    theta_ap = params[:, PC_THETA : PC_THETA + L]
    coef_ap = params[:, PC_COEF : PC_COEF + 1]
    scal_ap = params[:, PC_SCAL : PC_SCAL + 1]

    # dummy Sin with no input deps: forces the trig_and_small ACT table
    # (sin + square) to load immediately, overlapped with the input DMAs.
    scrd = wpool.tile([1, 2], F32, tag="scrd")
    nc.vector.memset(scrd[0:1, 0:1], 0.0)
    nc.scalar.activation(scrd[0:1, 1:2], scrd[0:1, 0:1], AF.Sin)

    # broadcast inds to all partitions via a K=1 f32r ones-matmul
    ones_row = wpool.tile([1, J], F32, tag="ones_row")
    nc.gpsimd.memset(ones_row[:], 1.0)
    inds_ps = bpool.tile([J, BLOC * L], F32, tag="inds_ps")
    nc.tensor.matmul(inds_ps[:], ones_row[:].bitcast(F32R), inds_row[:])

    # --- stage A: ARG[j, (i,l)] = theta[j,l] + scal[j]*inds[i,l]   (f32)
    arg = wpool.tile([J, BLOC * L], F32, tag="arg")
    in_bc = inds_ps[:].rearrange("j (i l) -> j i l", i=BLOC, l=L)
    th_bc = theta_ap.unsqueeze(1).broadcast_to([J, BLOC, L])
    arg_v = arg[:].rearrange("j (i l) -> j i l", i=BLOC, l=L)
    nc.vector.scalar_tensor_tensor(
        out=arg_v, in0=in_bc, scalar=scal_ap, in1=th_bc,
        op0=ALU.mult, op1=ALU.add,
    )

    # --- CS[j, (t,i,l)]: t=0 -> cos(A), t=1 -> sin(A); bf16 out
    # cos(A) = sin(pi/2 - A); A in (-1, 2.58) keeps both args in [-pi, pi].
    cs = wpool.tile([J, 2 * BLOC * L], BF16, tag="cs")
    nc.scalar.activation(
        cs[:, 0 : BLOC * L], arg[:], AF.Sin,
        bias=params[:, PC_HALFPI : PC_HALFPI + 1], scale=-1.0,
    )
    nc.scalar.activation(cs[:, BLOC * L : 2 * BLOC * L], arg[:], AF.Sin)

    # fold coef into site l=0 (both branches) => every T1 combo of group 0
    # carries exactly one coef_j factor.
    cs_v = cs[:].rearrange("j (t i l) -> j t i l", t=2, i=BLOC, l=L)
    nc.vector.tensor_scalar_mul(cs_v[:, :, :, 0:1], cs_v[:, :, :, 0:1], coef_ap)

    fin = wpool.tile([J, FIN_COLS], F32, tag="fin")
    # S-mode writes only partition 0 of its norm col; zero-fill the rest
    nc.gpsimd.memset(fin[:], 0.0)

    # --- doubling: L1 (site pairs, 4 combos), L2 (quads, 16 combos)
    # group 0 on DVE, group 1 on Pool (independent chains).
    eng = {0: nc.vector, 1: nc.gpsimd if G1_ENG == "pool" else nc.vector}
    l1 = [wpool.tile([J, BLOC * 16], BF16, tag=f"l1_{g}", name=f"l1_{g}") for g in range(2)]
    l2 = [wpool.tile([J, BLOC * 32], BF16, tag=f"l2_{g}", name=f"l2_{g}") for g in range(2)]
    for g in range(2):
        lo = g * 8  # first site of the group
        # L1[j, i, s, t1, t2] = CS[j,t1,i,lo+2s] * CS[j,t2,i,lo+2s+1]
        o1all = l1[g][:].rearrange(
            "j (i s t1 t2) -> j i s t1 t2", i=BLOC, s=4, t1=2, t2=2
        )
        for t1 in range(2):
            in1 = (
                cs_v[:, t1, :, lo : lo + 8 : 2]
                .unsqueeze(3)
                .broadcast_to([J, BLOC, 4, 2])
            )
            in2 = cs_v[:, :, :, lo + 1 : lo + 8 : 2].transpose([0, 2, 3, 1])
            o1 = o1all[:, :, :, t1, :]
            eng[g].tensor_tensor(out=o1, in0=in1, in1=in2, op=ALU.mult)
        # L2[j, i, d, q1, q2] = L1[j,i,2d,q1] * L1[j,i,2d+1,q2]; instr per d
        l1v = l1[g][:].rearrange("j (i s c) -> j i s c", i=BLOC, s=4, c=4)
        o2all = l2[g][:].rearrange(
            "j (i d q1 q2) -> j i d q1 q2", i=BLOC, d=2, q1=4, q2=4
        )
        for d in range(2):
            in1 = l1v[:, :, 2 * d, :].unsqueeze(3).broadcast_to([J, BLOC, 4, 4])
            in2 = l1v[:, :, 2 * d + 1, :].unsqueeze(2).broadcast_to([J, BLOC, 4, 4])
            o2 = o2all[:, :, d, :, :]
            eng[g].tensor_tensor(out=o2, in0=in1, in1=in2, op=ALU.mult)

    # --- L3 chunked by batch; per-chunk U tiles so PE/consumers pipeline
    sq_modes = SQ_MODES
    assert len(sq_modes) == BLOC and set(sq_modes) <= set("ASQWH")
    i0 = 0
    for c, csz in enumerate(CHUNKS):
        cw = csz * 256
        uc = [
            wpool.tile([J, cw], BF16, tag=f"u_{g}_{c}", name=f"u_{g}_{c}")
            for g in range(2)
        ]
        for g in range(2):
            l2v = l2[g][:].rearrange(
                "j (i d c16) -> j i d c16", i=BLOC, d=2, c16=16
            )
            in1 = (
                l2v[:, i0 : i0 + csz, 0, :]
                .unsqueeze(3)
                .broadcast_to([J, csz, 16, 16])
            )
            in2 = (
                l2v[:, i0 : i0 + csz, 1, :]
                .unsqueeze(2)
                .broadcast_to([J, csz, 16, 16])
            )
            ov = uc[g][:].rearrange(
                "j (i u1 u2) -> j i u1 u2", i=csz, u1=16, u2=16
            )
            ueng = nc.gpsimd if (g == 1 and c >= len(CHUNKS) - N_UPOOL) else nc.vector
            ueng.tensor_tensor(out=ov, in0=in1, in1=in2, op=ALU.mult)

        # amp partials for this chunk (DVE): fin[:, 16+i] = cU1[j,i,0]*U2[j,i,0]
        u1v = uc[0][:].rearrange("j (i t) -> j i t", i=csz, t=256)
        u2v = uc[1][:].rearrange("j (i t) -> j i t", i=csz, t=256)
        nc.vector.tensor_tensor(
            out=fin[:, 16 + i0 : 16 + i0 + csz],
            in0=u1v[:, :, 0], in1=u2v[:, :, 0], op=ALU.mult,
        )

        # D matmuls + square-reduce for this chunk's batch elements
        for k in range(csz):
            i = i0 + k
            dt = dpool.tile([J, 512], F32, tag="D")
            rhs = uc[1][:, k * 256 : (k + 1) * 256]
            for h in range(2):
                lhsT = uc[0][:, k * 256 + h * 128 : k * 256 + (h + 1) * 128]
                nc.tensor.matmul(dt[:, h * 256 : (h + 1) * 256], lhsT, rhs)
            mode = sq_modes[i]
            acc = fin[:, i : i + 1]
            if mode == "A":
                # 1-pass: ACT square + accum straight from PSUM
                nc.scalar.activation(dt[:], dt[:], AF.Square, accum_out=acc)
            elif mode in ("S", "Q"):
                dsq = spool.tile([J, 512], BF16, tag="dsq", name=f"dsq_{i}")
                nc.scalar.activation(dsq[:], dt[:], AF.Square)
                if mode == "S":
                    # Pool full reduce (partitions+cols) -> partition 0 only
                    nc.gpsimd.tensor_reduce(
                        out=fin[0:1, i : i + 1], in_=dsq[:],
                        axis=mybir.AxisListType.XYZWC, op=ALU.add,
                    )
                else:
                    nc.vector.tensor_scalar(
                        out=dsq[:], in0=dsq[:], scalar1=1.0, scalar2=None,
                        op0=ALU.mult, op1=ALU.add, accum_out=acc,
                    )
            elif mode == "H":  # split: ACT cols 0:256, DVE cols 256:512
                assert i >= 14
                nc.scalar.activation(
                    dt[:, 0:256], dt[:, 0:256], AF.Square, accum_out=acc
                )
                dsq = spool.tile([J, 256], BF16, tag="dsqh", name=f"dsqh_{i}")
                nc.vector.tensor_copy(dsq[:], dt[:, 256:512])
                nc.vector.scalar_tensor_tensor(
                    out=dsq[:], in0=dsq[:], scalar=1.0, in1=dsq[:],
                    op0=ALU.mult, op1=ALU.mult,
                    accum_out=fin[:, 32 + (i - 14) : 33 + (i - 14)],
                )
            else:  # W: DVE copies, then squares+sums on DVE
                dsq = spool.tile([J, 512], BF16, tag="dsq", name=f"dsq_{i}")
                nc.vector.tensor_copy(dsq[:], dt[:])
                nc.vector.scalar_tensor_tensor(
                    out=dsq[:], in0=dsq[:], scalar=1.0, in1=dsq[:],
                    op0=ALU.mult, op1=ALU.mult, accum_out=acc,
                )
        i0 += csz
    assert i0 == BLOC

    nc.sync.dma_start(out_d[:, :], fin[:])


def make_in_maps(input_ds, theta, coef):
    input_ds = np.asarray(input_ds, dtype=np.float32)
    theta = np.asarray(theta, dtype=np.float32)
    coef = np.asarray(coef, dtype=np.float32)
    pr = build_params()
    pr[:, PC_THETA : PC_THETA + L] = theta.transpose(1, 2, 0).reshape(J, L)
    pr[:, PC_COEF] = coef.reshape(J)
    in_maps = []
    for c in range(NCORES):
        sl = np.ascontiguousarray(
            input_ds[c * BLOC : (c + 1) * BLOC, :].reshape(1, BLOC * L)
        )
        in_maps.append({"params": pr, "inds": sl})
    return in_maps


_NC_CACHE = None


def _get_program():
    global _NC_CACHE
    if _NC_CACHE is None:
        _NC_CACHE = build_program()
    return _NC_CACHE


def combine_outputs(results, theta, coef):
    """Host-side tail: partition sums, ln, mean, and regularization."""
    theta = np.asarray(theta, dtype=np.float32)
    coef = np.asarray(coef, dtype=np.float32)
    ln_sum = 0.0
    for c in range(NCORES):
        fin = np.asarray(results[c]["out"], dtype=np.float64)  # [J, 32]
        norm = fin[:, 0:16].sum(axis=0)  # [16]
        norm[14] += fin[:, 32].sum()
        norm[15] += fin[:, 33].sum()
        amp = fin[:, 16:32].sum(axis=0)  # [16]
        prob = amp * amp
        ln_sum += float(np.sum(np.log(prob + EPS * norm) - np.log(norm)))
    loss = -ln_sum / float(B)
    tf = theta.astype(np.float64)
    cf = coef.astype(np.float64)
    loss += REG_THETA_M * float(np.mean(np.var(tf, axis=1, ddof=1)))
    loss += REG_THETA_P * float(np.mean(np.var(tf, axis=2, ddof=1)))
    loss += REG_C * float(np.var(cf, ddof=1))
    return np.float32(loss)


def kernel(input_ds, theta, coef):
    from concourse.bass_utils import run_bass_kernel_spmd

    nc = _get_program()
    in_maps = make_in_maps(input_ds, theta, coef)
    res = run_bass_kernel_spmd(nc, in_maps, core_ids=list(range(NCORES)))
    return combine_outputs(res.results, theta, coef)


# revision 17
# speedup vs baseline: 1.0146x; 1.0146x over previous
"""Trainium2 Bass kernel for the MANTIS quantum-circuit-loss nn.Module.

Shapes (hardcoded): B=128, L=16, M=32, P=4.  8 NeuronCores, batch-sharded
(16 batch elements per core).

Math
----
Let j = (m, p) flattened (M*P = 128 == partition count) and
    A[b, l, j] = theta[l, j] + scal[p(j)] * input_ds[b, l]
    CA = cos(A), SA = sin(A)                       (ACT Sin + pi/2 bias)

prob term:      amp[b]  = sum_j coef_j prod_l CA[b,l,j]
normalization:  norm[b] = sum_{j,k} coef_j coef_k prod_l cos(A[b,l,j]-A[b,l,k])

Using cos(a-b) = cos a cos b + sin a sin b, norm[b] is the squared norm of a
sum of 128 product states in the 2^16-dim site space.  Split the 16 sites
into two groups of 8; for each group build the 256 branch-product vectors
    U_g[j, T] = prod_{l in g} X_{T_l}[b, l, j],  X_0 = CA, X_1 = SA
by log-doubling (elementwise multiplies, bf16).  With coef folded into U1:
    D_b[T1, T2] = sum_j (c U1)[j, T1] U2[j, T2]    (PE matmul, K = 128)
    norm[b] = sum_{T1,T2} D_b^2
    amp[b]  = D_b[0, 0]
The device emits per-(partition, b) partials fin[128, 32]
(cols 0:16 = norm partials, 16:32 = amp partials); the host finishes:
partition sums, ln, mean over b, and the tiny regularization variances.

input_ds rides in as a single [1, 256] row and is broadcast to all 128
partitions with a K=1 f32r ones-matmul (cheaper than DMAing the
broadcast).  Engine split: ACT does cos/sin + PSUM square passes; DVE
does stage A + group-0 doubling + its share of square passes; Pool
(nc.gpsimd) does group-1 doubling and reduce second-passes.
"""

import math
import os

import numpy as np

import concourse.bacc as bacc
import concourse.bass as bass
import concourse.mybir as mybir
import concourse.tile as tile

B, L, M, P = 128, 16, 32, 4
NCORES = 8
BLOC = B // NCORES  # 16 batch elements per core
J = M * P  # 128
EPS = 1e-20
REG_C = 0.01
REG_THETA_M = 0.01
REG_THETA_P = 0.01

F32 = mybir.dt.float32
F32R = mybir.dt.float32r
BF16 = mybir.dt.bfloat16
AF = mybir.ActivationFunctionType
ALU = mybir.AluOpType

# batch chunk sizes for the U-build / matmul / square pipeline
CHUNKS = [int(x) for x in os.environ.get("MANTIS_CHUNKS", "1,1,2,4,4,4").split(",")]
# per-b square-reduce mode (16 chars):
#   A = ACT Square+accum direct from PSUM (1 pass)
#   S = ACT Square -> SBUF bf16, Pool tensor_reduce XYZWC -> fin[0, i]
#   Q = ACT Square -> SBUF bf16, DVE sums (tensor_scalar accum)
#   W = DVE copy -> SBUF bf16, DVE squares+sums (STT accum)
#   H = column-split: ACT squares cols 0:256, DVE copy+squares cols 256:512
#       (second half accumulates into fin[:, 32 + (i-14)]; only i=14,15)
SQ_MODES = os.environ.get("MANTIS_SQ", "AAAQAAAQAAAQAAAA")
# how many U chunks of group 1 go to Pool (from chunk 0 upward)
N_UPOOL = int(os.environ.get("MANTIS_UPOOL", "0"))
# doubling group-1 engine: pool | dve
G1_ENG = os.environ.get("MANTIS_G1ENG", "pool")

# params column layout
PC_THETA = 0  # 16 cols: theta_t[j, l]
PC_COEF = 16  # 1 col
PC_SCAL = 17  # 1 col: pi / 2^(p(j)+1)
PC_HALFPI = 18  # 1 col: pi/2 (ACT bias for cos-via-sin)
P_COLS = 19

# fin layout: [J, 34]: 0:16 norm partials, 16:32 amp partials, 32:34 H halves
FIN_COLS = 34


def build_params() -> np.ndarray:
    pr = np.zeros((J, P_COLS), dtype=np.float32)
    sf = (np.pi / 2.0 ** (np.arange(P) + 1.0)).astype(np.float32)
    pr[:, PC_SCAL] = np.tile(sf, M)
    pr[:, PC_HALFPI] = np.pi / 2.0
    return pr


def build_program():
    """Build the SPMD Bass/Tile program (identical on all 8 cores)."""
    nc = bacc.Bacc(
        "TRN2",
        target_bir_lowering=False,
        debug=False,
        num_devices=NCORES,
    )
    params_d = nc.dram_tensor("params", [J, P_COLS], F32, kind="ExternalInput")
    inds_d = nc.dram_tensor("inds", [1, BLOC * L], F32R, kind="ExternalInput")
    out_d = nc.dram_tensor("out", [J, FIN_COLS], F32, kind="ExternalOutput")

    with tile.TileContext(nc) as tc:
        with (
            tc.tile_pool(name="const", bufs=1) as cpool,
            tc.tile_pool(name="work", bufs=1) as wpool,
            tc.tile_pool(name="dps", bufs=7, space=bass.MemorySpace.PSUM) as dpool,
            tc.tile_pool(name="bps", bufs=1, space=bass.MemorySpace.PSUM) as bpool,
            tc.tile_pool(name="dsqp", bufs=3) as spool,
        ):
            _emit(nc, tc, cpool, wpool, dpool, bpool, spool, params_d, inds_d, out_d)
    nc.compile()
    return nc


def _emit(nc, tc, cpool, wpool, dpool, bpool, spool, params_d, inds_d, out_d):
    params = cpool.tile([J, P_COLS], F32, tag="params")
    inds_row = cpool.tile([1, BLOC * L], F32R, tag="inds_row")
    nc.sync.dma_start(inds_row[:], inds_d[:, :])
    nc.sync.dma_start(params[:], params_d[:, :])

    theta_ap = params[:, PC_THETA : PC_THETA + L]
    coef_ap = params[:, PC_COEF : PC_COEF + 1]
    scal_ap = params[:, PC_SCAL : PC_SCAL + 1]

    # dummy Sin with no input deps: forces the trig_and_small ACT table
    # (sin + square) to load immediately, overlapped with the input DMAs.
    scrd = wpool.tile([1, 2], F32, tag="scrd")
    nc.vector.memset(scrd[0:1, 0:1], 0.0)
    nc.scalar.activation(scrd[0:1, 1:2], scrd[0:1, 0:1], AF.Sin)

    # broadcast inds to all partitions via a K=1 f32r ones-matmul
    ones_row = wpool.tile([1, J], F32, tag="ones_row")
    nc.gpsimd.memset(ones_row[:], 1.0)
    inds_ps = bpool.tile([J, BLOC * L], F32, tag="inds_ps")
    nc.tensor.matmul(inds_ps[:], ones_row[:].bitcast(F32R), inds_row[:])

    # --- stage A: ARG[j, (i,l)] = theta[j,l] + scal[j]*inds[i,l]   (f32)
    arg = wpool.tile([J, BLOC * L], F32, tag="arg")
    in_bc = inds_ps[:].rearrange("j (i l) -> j i l", i=BLOC, l=L)
    th_bc = theta_ap.unsqueeze(1).broadcast_to([J, BLOC, L])
    arg_v = arg[:].rearrange("j (i l) -> j i l", i=BLOC, l=L)
    nc.vector.scalar_tensor_tensor(
        out=arg_v, in0=in_bc, scalar=scal_ap, in1=th_bc,
        op0=ALU.mult, op1=ALU.add,
    )

    # --- CS[j, (t,i,l)]: t=0 -> cos(A), t=1 -> sin(A); bf16 out
    # cos(A) = sin(pi/2 - A); A in (-1, 2.58) keeps both args in [-pi, pi].
    cs = wpool.tile([J, 2 * BLOC * L], BF16, tag="cs")
    nc.scalar.activation(
        cs[:, 0 : BLOC * L], arg[:], AF.Sin,
        bias=params[:, PC_HALFPI : PC_HALFPI + 1], scale=-1.0,
    )
    nc.scalar.activation(cs[:, BLOC * L : 2 * BLOC * L], arg[:], AF.Sin)

    # fold coef into site l=0 (both branches) => every T1 combo of group 0
    # carries exactly one coef_j factor.
    cs_v = cs[:].rearrange("j (t i l) -> j t i l", t=2, i=BLOC, l=L)
    nc.vector.tensor_scalar_mul(cs_v[:, :, :, 0:1], cs_v[:, :, :, 0:1], coef_ap)

    fin = wpool.tile([J, FIN_COLS], F32, tag="fin")
    # S-mode writes only partition 0 of its norm col; zero-fill the rest
    nc.gpsimd.memset(fin[:], 0.0)

    # --- doubling: L1 (site pairs, 4 combos), L2 (quads, 16 combos)
    # group 0 on DVE, group 1 on Pool (independent chains).
    eng = {0: nc.vector, 1: nc.gpsimd if G1_ENG == "pool" else nc.vector}
    l1 = [wpool.tile([J, BLOC * 16], BF16, tag=f"l1_{g}", name=f"l1_{g}") for g in range(2)]
    l2 = [wpool.tile([J, BLOC * 32], BF16, tag=f"l2_{g}", name=f"l2_{g}") for g in range(2)]
    # two passes: batches 0:4 first so chunk-0/1/2 matmuls and ACT squares
    # start ~2us earlier; 4:16 follows while the pipeline drains.
    for (b0, b1) in ((0, 4), (4, BLOC)):
        nb = b1 - b0
        for g in range(2):
            lo = g * 8  # first site of the group
            # L1[j, i, s, t1, t2] = CS[j,t1,i,lo+2s] * CS[j,t2,i,lo+2s+1]
            o1all = l1[g][:].rearrange(
                "j (i s t1 t2) -> j i s t1 t2", i=BLOC, s=4, t1=2, t2=2
            )
            for t1 in range(2):
                in1 = (
                    cs_v[:, t1, b0:b1, lo : lo + 8 : 2]
                    .unsqueeze(3)
                    .broadcast_to([J, nb, 4, 2])
                )
                in2 = cs_v[:, :, b0:b1, lo + 1 : lo + 8 : 2].transpose([0, 2, 3, 1])
                o1 = o1all[:, b0:b1, :, t1, :]
                eng[g].tensor_tensor(out=o1, in0=in1, in1=in2, op=ALU.mult)
            # L2[j, i, d, q1, q2] = L1[j,i,2d,q1] * L1[j,i,2d+1,q2]
            l1v = l1[g][:].rearrange("j (i s c) -> j i s c", i=BLOC, s=4, c=4)
            o2all = l2[g][:].rearrange(
                "j (i d q1 q2) -> j i d q1 q2", i=BLOC, d=2, q1=4, q2=4
            )
            for d in range(2):
                in1 = l1v[:, b0:b1, 2 * d, :].unsqueeze(3).broadcast_to([J, nb, 4, 4])
                in2 = l1v[:, b0:b1, 2 * d + 1, :].unsqueeze(2).broadcast_to([J, nb, 4, 4])
                o2 = o2all[:, b0:b1, d, :, :]
                eng[g].tensor_tensor(out=o2, in0=in1, in1=in2, op=ALU.mult)

    # --- L3 chunked by batch; per-chunk U tiles so PE/consumers pipeline
    sq_modes = SQ_MODES
    assert len(sq_modes) == BLOC and set(sq_modes) <= set("ASQWH")
    i0 = 0
    for c, csz in enumerate(CHUNKS):
        cw = csz * 256
        uc = [
            wpool.tile([J, cw], BF16, tag=f"u_{g}_{c}", name=f"u_{g}_{c}")
            for g in range(2)
        ]
        for g in range(2):
            l2v = l2[g][:].rearrange(
                "j (i d c16) -> j i d c16", i=BLOC, d=2, c16=16
            )
            in1 = (
                l2v[:, i0 : i0 + csz, 0, :]
                .unsqueeze(3)
                .broadcast_to([J, csz, 16, 16])
            )
            in2 = (
                l2v[:, i0 : i0 + csz, 1, :]
                .unsqueeze(2)
                .broadcast_to([J, csz, 16, 16])
            )
            ov = uc[g][:].rearrange(
                "j (i u1 u2) -> j i u1 u2", i=csz, u1=16, u2=16
            )
            ueng = nc.gpsimd if (g == 1 and c >= len(CHUNKS) - N_UPOOL) else nc.vector
            ueng.tensor_tensor(out=ov, in0=in1, in1=in2, op=ALU.mult)

        # amp partials for this chunk (DVE): fin[:, 16+i] = cU1[j,i,0]*U2[j,i,0]
        u1v = uc[0][:].rearrange("j (i t) -> j i t", i=csz, t=256)
        u2v = uc[1][:].rearrange("j (i t) -> j i t", i=csz, t=256)
        nc.vector.tensor_tensor(
            out=fin[:, 16 + i0 : 16 + i0 + csz],
            in0=u1v[:, :, 0], in1=u2v[:, :, 0], op=ALU.mult,
        )

        # D matmuls + square-reduce for this chunk's batch elements
        for k in range(csz):
            i = i0 + k
            dt = dpool.tile([J, 512], F32, tag="D")
            rhs = uc[1][:, k * 256 : (k + 1) * 256]
            for h in range(2):
                lhsT = uc[0][:, k * 256 + h * 128 : k * 256 + (h + 1) * 128]
                nc.tensor.matmul(dt[:, h * 256 : (h + 1) * 256], lhsT, rhs)
            mode = sq_modes[i]
            acc = fin[:, i : i + 1]
            if mode == "A":
                # 1-pass: ACT square + accum straight from PSUM
                nc.scalar.activation(dt[:], dt[:], AF.Square, accum_out=acc)
            elif mode in ("S", "Q"):
                dsq = spool.tile([J, 512], BF16, tag="dsq", name=f"dsq_{i}")
                nc.scalar.activation(dsq[:], dt[:], AF.Square)
                if mode == "S":
                    # Pool full reduce (partitions+cols) -> partition 0 only
                    nc.gpsimd.tensor_reduce(
                        out=fin[0:1, i : i + 1], in_=dsq[:],
                        axis=mybir.AxisListType.XYZWC, op=ALU.add,
                    )
                else:
                    nc.vector.tensor_scalar(
                        out=dsq[:], in0=dsq[:], scalar1=1.0, scalar2=None,
                        op0=ALU.mult, op1=ALU.add, accum_out=acc,
                    )
            elif mode == "H":  # split: ACT cols 0:256, DVE cols 256:512
                assert i >= 14
                nc.scalar.activation(
                    dt[:, 0:256], dt[:, 0:256], AF.Square, accum_out=acc
                )
                dsq = spool.tile([J, 256], BF16, tag="dsqh", name=f"dsqh_{i}")
                nc.vector.tensor_copy(dsq[:], dt[:, 256:512])
                nc.vector.scalar_tensor_tensor(
                    out=dsq[:], in0=dsq[:], scalar=1.0, in1=dsq[:],
                    op0=ALU.mult, op1=ALU.mult,
                    accum_out=fin[:, 32 + (i - 14) : 33 + (i - 14)],
                )
            else:  # W: DVE copies, then squares+sums on DVE
                dsq = spool.tile([J, 512], BF16, tag="dsq", name=f"dsq_{i}")
                nc.vector.tensor_copy(dsq[:], dt[:])
                nc.vector.scalar_tensor_tensor(
                    out=dsq[:], in0=dsq[:], scalar=1.0, in1=dsq[:],
                    op0=ALU.mult, op1=ALU.mult, accum_out=acc,
                )
        i0 += csz
    assert i0 == BLOC

    nc.sync.dma_start(out_d[:, :], fin[:])


def make_in_maps(input_ds, theta, coef):
    input_ds = np.asarray(input_ds, dtype=np.float32)
    theta = np.asarray(theta, dtype=np.float32)
    coef = np.asarray(coef, dtype=np.float32)
    pr = build_params()
    pr[:, PC_THETA : PC_THETA + L] = theta.transpose(1, 2, 0).reshape(J, L)
    pr[:, PC_COEF] = coef.reshape(J)
    in_maps = []
    for c in range(NCORES):
        sl = np.ascontiguousarray(
            input_ds[c * BLOC : (c + 1) * BLOC, :].reshape(1, BLOC * L)
        )
        in_maps.append({"params": pr, "inds": sl})
    return in_maps


_NC_CACHE = None


def _get_program():
    global _NC_CACHE
    if _NC_CACHE is None:
        _NC_CACHE = build_program()
    return _NC_CACHE


def combine_outputs(results, theta, coef):
    """Host-side tail: partition sums, ln, mean, and regularization."""
    theta = np.asarray(theta, dtype=np.float32)
    coef = np.asarray(coef, dtype=np.float32)
    ln_sum = 0.0
    for c in range(NCORES):
        fin = np.asarray(results[c]["out"], dtype=np.float64)  # [J, 32]
        norm = fin[:, 0:16].sum(axis=0)  # [16]
        norm[14] += fin[:, 32].sum()
        norm[15] += fin[:, 33].sum()
        amp = fin[:, 16:32].sum(axis=0)  # [16]
        prob = amp * amp
        ln_sum += float(np.sum(np.log(prob + EPS * norm) - np.log(norm)))
    loss = -ln_sum / float(B)
    tf = theta.astype(np.float64)
    cf = coef.astype(np.float64)
    loss += REG_THETA_M * float(np.mean(np.var(tf, axis=1, ddof=1)))
    loss += REG_THETA_P * float(np.mean(np.var(tf, axis=2, ddof=1)))
    loss += REG_C * float(np.var(cf, ddof=1))
    return np.float32(loss)


def kernel(input_ds, theta, coef):
    from concourse.bass_utils import run_bass_kernel_spmd

    nc = _get_program()
    in_maps = make_in_maps(input_ds, theta, coef)
    res = run_bass_kernel_spmd(nc, in_maps, core_ids=list(range(NCORES)))
    return combine_outputs(res.results, theta, coef)
